# revision 32
# baseline (speedup 1.0000x reference)
"""DiffAttn Trainium2 kernel (8-core SPMD, no collectives) — v5.

Problem: B=2, T=2048, IN_DIM=OUT_DIM=1024, H=8 v-heads (2D=64), 2H=16 qk-heads
(D=32).  Core c = 4*b + g handles batch b, head-group g: qk-heads {4g..4g+3},
v-heads {2g, 2g+1}, all T queries of its batch.  Out-projection row-sharded;
host sums 4 partials per batch (bf16 partials; host upcasts and sums fp32).

Design notes (measured ~231us vs 242.5us for v2):
- The Act engine's exp stream (128 x [128,1024] ~1.12us each) is the pacing
  floor; PSUM (8 banks: accA+accB 4, "d" ring 2x4KB) limits Act lookahead
  to one tile, so every engine must keep dots flowing on schedule.
- In-order engine queues are the main hazard: never put a blocked
  instruction ahead of ready work.  dots are issued first each step;
  boundary drains stay off the Act queue mid-run; a heavy deferred
  projection borrows its step's attn@v slot (attn@v is elastic via the
  16-deep e ring).
- The PE DVFS clock (2.4 -> 1.2 GHz) drops on idle windows and everything
  downstream slows: warmup matmuls ramp it during the input DMA, qT2/qT3
  projections are scheduled INTO the qb-boundary holds, and dummy matmuls
  into the freed acc banks keep it up through the tail out-projection.
- Phase C: denominator rows (PSUM partition 64) land on partition 0 of one
  [1,2048] SBUF row via cross-partition DVE copies (legal when the start
  partition is 32-aligned; PSUM reads must also start 32-aligned and
  GPSIMD cannot touch PSUM at all).  r2/rinv fan out through tiny K=1/K=2
  PE matmuls against ones/mask stationaries (the old sA/sB DMA fan and 4
  gpsimd partition_broadcasts are gone).  ss is ONE matmul with a [128,2]
  ones stationary -> [2,512]; Ln/Exp run at 512-free; Square rides DVE
  (Act for the last qb); comb's bottom half is written cross-partition
  (no SBUF->SBUF DMA hop); the last qb's drains ride the idle Act engine
  and attn@v for its final key-tiles issues with no lag.
"""
import math

import numpy as np

H = 8
D = 32
LAMBDA_INIT = 0.8 - 0.6 * math.exp(-0.3)
B, T, IN_DIM, OUT_DIM = 2, 2048, 1024, 1024
E = 2 * H * D  # 512

N_CORES = 8
GROUPS = 4            # head groups (cores per batch)
QB = 512              # query block
NQB = T // QB         # 4
KT = 128              # key tile (PSUM partition dim for dots)
NKT = T // KT         # 16
NIN = IN_DIM // 128   # 8
LAG = 4               # attn@v issue lag in key-tiles
WARM = 10             # PE warmup matmuls (clock ramp during x DMA)

_compiled = None


def _build():
    import concourse.bass as bass  # noqa: F401
    import concourse.mybir as mybir
    from concourse import bacc
    from concourse.tile import TileContext

    f32 = mybir.dt.float32
    bf16 = mybir.dt.bfloat16
    AF = mybir.ActivationFunctionType
    MUL = mybir.AluOpType.mult

    nc = bacc.Bacc("TRN2", target_bir_lowering=False, num_devices=N_CORES)

    xt = nc.dram_tensor("xt", [4 * 128, NIN * 512], bf16, kind="ExternalInput")
    wq = nc.dram_tensor("wq", [128, NIN * 128], bf16, kind="ExternalInput")
    wk = nc.dram_tensor("wk", [128, NIN * 128], bf16, kind="ExternalInput")
    wv = nc.dram_tensor("wv", [128, NIN * 128], bf16, kind="ExternalInput")
    wo = nc.dram_tensor("wo", [128, OUT_DIM], bf16, kind="ExternalInput")
    lam2 = nc.dram_tensor("lam2", [2, 1], f32, kind="ExternalInput")
    gamp = nc.dram_tensor("gamp", [128, 1], f32, kind="ExternalInput")
    ones2 = nc.dram_tensor("ones2", [128, 2], bf16, kind="ExternalInput")
    bc = nc.dram_tensor("bc", [2, 128], bf16, kind="ExternalInput")
    idn = nc.dram_tensor("idn", [128, 128], bf16, kind="ExternalInput")
    outT = nc.dram_tensor("outT", [4 * 128, 8 * 512], bf16,
                          kind="ExternalOutput")

    with TileContext(nc) as tc:
        with tc.tile_pool(name="persist", bufs=1) as pp:
            # ---- persistent SBUF ----
            wq_sb = pp.tile([128, NIN * 128], bf16)
            wk_sb = pp.tile([128, NIN * 128], bf16)
            wv_sb = pp.tile([128, NIN * 128], bf16)
            wo_sb = pp.tile([128, OUT_DIM], bf16)
            xT_sb = pp.tile([128, 4, NIN * 512], bf16)
            # per-token-block tiles: deferred projections write only their
            # own block, so dots/attn@v never pick up false whole-tile deps
            qT_tb = [pp.tile([128, 512], bf16, name=f"qT{t}") for t in range(4)]
            kT_tb = [pp.tile([128, 512], bf16, name=f"kT{t}") for t in range(4)]
            vT_tb = [pp.tile([128, 512], bf16, name=f"vT{t}") for t in range(4)]
            v_tb = [pp.tile([128, 4, 130], bf16, name=f"v{t}") for t in range(4)]
            lam_sb = pp.tile([2, 1], f32)
            gam_sb = pp.tile([128, 1], f32)
            ones2_sb = pp.tile([128, 2], bf16)
            bc_sb = pp.tile([2, 128], bf16)
            idn_sb = pp.tile([128, 128], bf16)
            warm = pp.tile([128, 512], bf16)
            warm_e = pp.tile([128, 512], bf16)

            # x tb0 right after the k/q weights so the first projections can
            # start ~4us in; the rest interleaved with the small consts
            nc.sync.dma_start(out=wk_sb[:, :], in_=wk[:, :])
            nc.sync.dma_start(out=wq_sb[:, :], in_=wq[:, :])
            nc.sync.dma_start(out=xT_sb[:, 0, :], in_=xt[0:128, :])
            nc.sync.dma_start(out=idn_sb[:, :], in_=idn[:, :])
            nc.sync.dma_start(out=wv_sb[:, :], in_=wv[:, :])
            nc.sync.dma_start(out=xT_sb[:, 1, :], in_=xt[128:256, :])
            nc.sync.dma_start(out=lam_sb[:, :], in_=lam2[:, :])
            nc.sync.dma_start(out=gam_sb[:, :], in_=gamp[:, :])
            nc.sync.dma_start(out=ones2_sb[:, :], in_=ones2[:, :])
            nc.sync.dma_start(out=bc_sb[:, :], in_=bc[:, :])
            nc.sync.dma_start(out=wo_sb[:, :], in_=wo[:, :])
            nc.sync.dma_start(out=xT_sb[:, 2, :], in_=xt[256:384, :])
            nc.sync.dma_start(out=xT_sb[:, 3, :], in_=xt[384:512, :])

            # ---- phase A: warmup + projections ----
            with tc.tile_pool(name="psA", bufs=1, space="PSUM") as psA:
                nc.vector.memset(warm[:, :], 0.0)
                wm = psA.tile([128, 512], f32, tag="warm", bufs=1)
                for _ in range(WARM):
                    nc.tensor.matmul(wm[:, :], warm[:, :128], warm[:, :],
                                     start=True, stop=True)
                # natural_log_exp_and_others holds Exp+Copy+Ln: one load
                # up front keeps the Act table warm for the whole kernel
                nc.scalar.add_instruction(mybir.InstLoadActFuncSet(
                    name=nc.scalar.bass.get_next_instruction_name(),
                    ins=[], outs=[], act_func_set_id=6))
                nc.scalar.activation(warm_e[:, :], wm[:, :], AF.Exp)

                # minimal prefix: only what dots(qb0,kt0) needs — the kT0
                # and qT0 projections.  vT0 + its transposes are deferred
                # into the main loop (steps 0-1), ahead of attn@v(0) at
                # step LAG.  First exp ~14us instead of ~24us.
                for dst, w_sb in ((kT_tb[0], wk_sb), (qT_tb[0], wq_sb)):
                    p = psA.tile([128, 512], f32, tag="proj", bufs=2)
                    for c in range(NIN):
                        nc.tensor.matmul(
                            p[:, :], w_sb[:, 128 * c:128 * (c + 1)],
                            xT_sb[:, 0, 512 * c:512 * (c + 1)],
                            start=(c == 0), stop=(c == NIN - 1))
                    nc.vector.tensor_copy(dst[:, :], p[:, :])
                with tc.tile_pool(name="onescr", bufs=1) as op_:
                    oscr = op_.tile([128, 4], f32)
                    nc.vector.memset(oscr[:, :], 1.0)
                    for t in range(4):
                        nc.vector.tensor_copy(
                            v_tb[t][:, :, 64:65].rearrange("p n 1 -> p n"),
                            oscr[:, :])
                        nc.vector.tensor_copy(
                            v_tb[t][:, :, 129:130].rearrange("p n 1 -> p n"),
                            oscr[:, :])

            # ---- phase B + C interleaved ----
            with (
                tc.tile_pool(name="mp", bufs=1, space="PSUM") as mp,
                tc.tile_pool(name="ep", bufs=1) as ep,
                tc.tile_pool(name="cp", bufs=1) as cp,
            ):
                es = {}
                accs = {}
                pend_attnv = []   # (qb, kt) awaiting attn@v issue
                chunks = []       # pending phase-C closures
                hold = [0]        # skip-pops pending after a boundary

                def a_proj(dst, w_sb, tb, nm=[0], lo=0, hi=512):
                    nm[0] += 1
                    pname = f"aproj{nm[0]}"
                    w = hi - lo
                    def f():
                        p = mp.tile([128, 1024], f32, tag="d",
                                    name=pname, bufs=2)
                        for c in range(NIN):
                            nc.tensor.matmul(
                                p[:, 0:w], w_sb[:, 128 * c:128 * (c + 1)],
                                xT_sb[:, tb, 512 * c + lo:512 * c + hi],
                                start=(c == 0), stop=(c == NIN - 1))
                        nc.vector.tensor_copy(dst[tb][:, lo:hi], p[:, 0:w])
                    return f

                def a_transp(tb):
                    def f():
                        tp = mp.tile([128, 2048], bf16, tag="d",
                                     name=f"atr{tb}", bufs=2)
                        for j in range(4):
                            nc.tensor.transpose(
                                tp[:, 128 * j:128 * (j + 1)],
                                vT_tb[tb][:, 128 * j:128 * (j + 1)],
                                idn_sb[:, :])
                        for j in range(4):
                            nc.vector.tensor_copy(
                                v_tb[tb][:, j, 0:64],
                                tp[:, 128 * j:128 * j + 64])
                            nc.vector.tensor_copy(
                                v_tb[tb][:, j, 65:129],
                                tp[:, 128 * j + 64:128 * (j + 1)])
                    return f

                # (fn, is_heavy): heavy projections borrow the step's
                # attn@v PE slot; transposes are cheap enough not to.
                # qT2/qT3 sit ON the qb boundaries (steps 16/32) where the
                # hold idles the PE anyway — free real estate that also
                # keeps the DVFS clock up.
                awork = {
                    0: (a_proj(vT_tb, wv_sb, 0), True),
                    1: (a_transp(0), False),
                    2: (a_proj(kT_tb, wk_sb, 1), True),
                    4: (a_proj(vT_tb, wv_sb, 1), True),
                    5: (a_transp(1), False),
                    6: (a_proj(kT_tb, wk_sb, 2), True),
                    7: (a_proj(vT_tb, wv_sb, 2), True),
                    8: (a_transp(2), False),
                    9: (a_proj(kT_tb, wk_sb, 3), True),
                    10: (a_proj(vT_tb, wv_sb, 3), True),
                    11: (a_transp(3), False),
                    13: (a_proj(qT_tb, wq_sb, 1), True),
                    # qT2/qT3 in N=256 halves: one monolithic 8-matmul
                    # projection overruns a step's PE slack and delays the
                    # next dots (~4us exp gap); halves fit the boundary
                    # hold's slack
                    18: (a_proj(qT_tb, wq_sb, 2, lo=0, hi=256), False),
                    20: (a_proj(qT_tb, wq_sb, 2, lo=256, hi=512), False),
                    34: (a_proj(qT_tb, wq_sb, 3, lo=0, hi=256), False),
                    36: (a_proj(qT_tb, wq_sb, 3, lo=256, hi=512), False),
                }

                def issue_attnv(qb, kt):
                    if kt == 0:
                        accs[(qb, 0)] = mp.tile([65, 1024], f32, tag="accA",
                                                name=f"accA{qb}", bufs=1)
                        accs[(qb, 1)] = mp.tile([65, 1024], f32, tag="accB",
                                                name=f"accB{qb}", bufs=1)
                    for vh in range(2):
                        e = es.pop((qb, kt, vh))
                        for hh in range(2):
                            nc.tensor.matmul(
                                accs[(qb, vh)][:, 512 * hh:512 * (hh + 1)],
                                v_tb[kt // 4][:, kt % 4, 65 * vh:65 * (vh + 1)],
                                e[:, 512 * hh:512 * (hh + 1)],
                                start=(kt == 0), stop=(kt == NKT - 1))

                def make_chunks(qb):
                    aA, aB = accs.pop((qb, 0)), accs.pop((qb, 1))
                    last = qb == NQB - 1
                    a_sbA = cp.tile([64, 1024], f32, tag="asbA",
                                    name=f"asbA{qb}", bufs=2)
                    a_sbB = cp.tile([64, 1024], f32, tag="asbB",
                                    name=f"asbB{qb}", bufs=2)
                    srS = cp.tile([1, 2048], f32, tag="srS", name=f"srS{qb}",
                                  bufs=2)
                    # boundary drains (eager, so the acc banks free fast).
                    # denominator rows (PSUM partition 64) land on
                    # partition 0 via cross-partition copies.  Mid-run all
                    # four ride the DVE — putting any on Act would block
                    # the in-order exp stream behind attnv(qb,15).  Rows
                    # first so the ratio chain starts at +2.3us.  On the
                    # last qb the exp stream is over: Act takes the B side
                    # and the drains pair up.
                    nc.vector.tensor_copy(srS[0:1, 0:1024], aA[64:65, :])
                    if last:
                        nc.scalar.copy(srS[0:1, 1024:2048], aB[64:65, :])
                        nc.vector.tensor_copy(a_sbA[:, :], aA[0:64, :])
                        nc.scalar.copy(a_sbB[:, :], aB[0:64, :])
                    else:
                        nc.vector.tensor_copy(srS[0:1, 1024:2048],
                                              aB[64:65, :])
                        nc.vector.tensor_copy(a_sbA[:, :], aA[0:64, :])
                        nc.vector.tensor_copy(a_sbB[:, :], aB[0:64, :])

                    st = {}
                    nwt = [0]

                    def c_rcp():
                        rr = cp.tile([1, 1024], f32, tag="rr", bufs=2)
                        nc.vector.reciprocal_approx_fast(
                            out=rr[0:1, 0:512], in_=srS[0:1, 512:1024])
                        nc.vector.reciprocal_approx_fast(
                            out=rr[0:1, 512:1024], in_=srS[0:1, 1536:2048])
                        st["rr"] = rr

                    def c_r2():
                        r2 = cp.tile([1, 1024], bf16, tag="r2", bufs=2)
                        nc.vector.scalar_tensor_tensor(
                            r2[0:1, 0:512], srS[0:1, 0:512],
                            lam_sb[0:1, 0:1], st["rr"][0:1, 0:512],
                            op0=MUL, op1=MUL)
                        nc.vector.scalar_tensor_tensor(
                            r2[0:1, 512:1024], srS[0:1, 1024:1536],
                            lam_sb[0:1, 0:1], st["rr"][0:1, 512:1024],
                            op0=MUL, op1=MUL)
                        st["r2"] = r2

                    def tail_warm(n):
                        # last-qb only: dummy matmuls into the freed acc
                        # banks keep the PE clock from dropping before the
                        # out-projection
                        if not last:
                            return
                        wt = mp.tile([65, 512], f32, tag="accA",
                                     name=f"wt{qb}_{nwt[0]}", bufs=1)
                        nwt[0] += 1
                        for _ in range(n):
                            nc.tensor.matmul(wt[0:128 - 64, :],
                                             warm[:, 0:64], warm[:, :],
                                             start=True, stop=True)

                    def c_rb():
                        # broadcast the r2 row (partition 0) to 64
                        # partitions with two K=1 matmuls (ones stationary)
                        rb = mp.tile([64, 1024], f32, tag="d",
                                     name=f"rb{qb}", bufs=2)
                        for half in range(2):
                            nc.tensor.matmul(
                                rb[:, 512 * half:512 * (half + 1)],
                                bc_sb[0:1, 0:64],
                                st["r2"][0:1, 512 * half:512 * (half + 1)],
                                start=True, stop=True)
                        st["rb"] = rb
                        tail_warm(5)

                    def c_comb0():
                        t2 = cp.tile([64, 512], f32, tag="t2a", bufs=1)
                        comb = cp.tile([128, 512], f32, tag="comb",
                                       name=f"comb{qb}", bufs=2)
                        nc.vector.tensor_mul(t2[:, :],
                                             a_sbA[0:64, 512:1024],
                                             st["rb"][:, 0:512])
                        nc.vector.tensor_sub(comb[0:64, :],
                                             a_sbA[0:64, 0:512], t2[:, :])
                        st["comb"] = comb

                    def c_comb1():
                        # cross-partition write: in p0-63, out p64-127
                        # (legal: start partitions are 32-aligned) — no
                        # SBUF->SBUF DMA hop needed for the bottom half.
                        # On the last qb the pair rides GpSimd so the A
                        # and B halves run in parallel in the tail.
                        t2 = cp.tile([64, 512], f32, tag="t2b", bufs=1)
                        nc.vector.tensor_mul(t2[:, :],
                                             a_sbB[0:64, 512:1024],
                                             st["rb"][:, 512:1024])
                        nc.vector.tensor_sub(st["comb"][64:128, :],
                                             a_sbB[0:64, 0:512], t2[:, :])

                    def c_sq():
                        sq = cp.tile([128, 512], bf16, tag="sq", bufs=2)
                        if last:
                            nc.scalar.activation(sq[:, :], st["comb"][:, :],
                                                 AF.Square)
                        else:
                            nc.vector.tensor_mul(sq[:, :], st["comb"][:, :],
                                                 st["comb"][:, :])
                        st["sq"] = sq

                    def c_ss():
                        sst = mp.tile([2, 512], f32, tag="d",
                                      name=f"ss{qb}", bufs=2)
                        nc.tensor.matmul(sst[:, :], ones2_sb[:, 0:2],
                                         st["sq"][:, :], start=True, stop=True)
                        st["ss"] = sst
                        tail_warm(4)

                    def c_rinv():
                        rln = cp.tile([2, 512], f32, tag="rln", bufs=2)
                        nc.scalar.activation(rln[:, :], st["ss"][0:2, :],
                                             AF.Ln, scale=1.0 / 64.0)
                        rinv = cp.tile([2, 512], bf16, tag="rinv", bufs=2)
                        nc.scalar.activation(rinv[:, :], rln[:, :], AF.Exp,
                                             scale=-0.5)
                        st["rinv"] = rinv

                    def c_rb2():
                        # broadcast rinv rows (partitions 0/1) to 128
                        # partitions: top 64 get row 0, bottom 64 row 1
                        rb2 = mp.tile([128, 512], f32, tag="d",
                                      name=f"rb2_{qb}", bufs=2)
                        nc.tensor.matmul(rb2[:, :], bc_sb[0:2, 0:128],
                                         st["rinv"][0:2, :],
                                         start=True, stop=True)
                        st["rb2"] = rb2
                        tail_warm(4)

                    def c_finl():
                        finl = cp.tile([128, 512], bf16, tag="finl",
                                       name=f"finl{qb}", bufs=2)
                        nc.vector.scalar_tensor_tensor(
                            finl[:, :], st["comb"][:, :], gam_sb[:, 0:1],
                            st["rb2"][:, :], op0=MUL, op1=MUL)
                        st["finl"] = finl

                    def c_opj(p):
                        def f():
                            opj = mp.tile([128, 1024], f32, tag="d",
                                          name=f"opj{qb}_{p}", bufs=2)
                            for j in range(2):
                                oc = 2 * p + j
                                nc.tensor.matmul(
                                    opj[:, 512 * j:512 * (j + 1)],
                                    wo_sb[:, 128 * oc:128 * (oc + 1)],
                                    st["finl"][:, :], start=True, stop=True)
                            ostg = cp.tile([128, 1024], bf16, tag="ostg",
                                           name=f"ostg{qb}_{p}", bufs=2)
                            if last and p % 2 == 1:
                                nc.scalar.copy(ostg[:, :], opj[:, :])
                            else:
                                nc.vector.tensor_copy(ostg[:, :], opj[:, :])
                            nc.sync.dma_start(
                                out=outT[128 * qb:128 * (qb + 1),
                                         1024 * p:1024 * (p + 1)],
                                in_=ostg[:, :])
                        return f

                    return [c_rcp, c_r2, c_rb, c_comb0, c_comb1, c_sq,
                            c_ss, c_rinv, c_rb2, c_finl,
                            c_opj(0), None, c_opj(1), None,
                            c_opj(2), None, c_opj(3)]

                for qb in range(NQB):
                    for kt in range(NKT):
                        # all four dots back-to-back into 4 distinct PSUM
                        # banks -> 4-band row-tile concurrency on the PE
                        ds = [mp.tile([128, 1024], f32, tag="d",
                                      name=f"d{qb}_{kt}_{vh}", bufs=2)
                              for vh in range(2)]
                        for h in range(4):
                            nc.tensor.matmul(
                                ds[h // 2][:, 512 * (h % 2):512 * (h % 2 + 1)],
                                kT_tb[kt // 4][32 * h:32 * (h + 1),
                                               KT * (kt % 4):KT * (kt % 4 + 1)],
                                qT_tb[qb][32 * h:32 * (h + 1), :],
                                start=True, stop=True,
                                tile_position=(32 * h, 0))
                        for vh in range(2):
                            e = ep.tile([128, 1024], bf16, tag="e",
                                        name=f"e{qb}_{kt}_{vh}", bufs=20)
                            nc.scalar.activation(e[:, :], ds[vh][:, :], AF.Exp)
                            es[(qb, kt, vh)] = e
                        pend_attnv.append((qb, kt))
                        aw = awork.pop(16 * qb + kt, None)
                        if aw is not None:
                            aw[0]()
                            # a heavy projection borrows this step's
                            # attn@v PE slot (attn@v is elastic via the
                            # e-buf ring; delayed dots would gap the exps)
                            if aw[1] and not hold[0]:
                                hold[0] = 1
                        # final qb: no more exps to pace — issue attn@v
                        # immediately so the tail drain shrinks
                        lag_now = 0 if qb == NQB - 1 and kt >= NKT - 3 \
                            else LAG
                        if hold[0]:
                            hold[0] -= 1
                        else:
                            npops = 0
                            while len(pend_attnv) > lag_now and npops < 2:
                                aqb, akt = pend_attnv.pop(0)
                                issue_attnv(aqb, akt)
                                npops += 1
                                if akt == NKT - 1:
                                    chunks.extend(make_chunks(aqb))
                                    # give the boundary drain two extra kt
                                    # before the next qb's first attn@v
                                    # needs the acc banks back (the DVE
                                    # drain chain is ~4.6us ~ 2 kt)
                                    hold[0] = 3
                                    break
                        npop = 2 if len(chunks) > 10 else 1
                        for _ in range(npop):
                            if chunks:
                                ck = chunks.pop(0)
                                if ck is not None:
                                    ck()
                # drain
                while pend_attnv:
                    aqb, akt = pend_attnv.pop(0)
                    issue_attnv(aqb, akt)
                    if akt == NKT - 1:
                        chunks.extend(make_chunks(aqb))
                while chunks:
                    ck = chunks.pop(0)
                    if ck is not None:
                        ck()

    nc.compile()
    return nc


def _get_compiled():
    global _compiled
    if _compiled is None:
        _compiled = _build()
    return _compiled


def make_in_maps(x, Wq, Wkv, Wout, lambda_q1, lambda_k1, lambda_q2, lambda_k2,
                 gamma):
    import ml_dtypes
    bf = ml_dtypes.bfloat16
    x = np.asarray(x, dtype=np.float32)
    Wq = np.asarray(Wq, dtype=np.float32)
    Wkv = np.asarray(Wkv, dtype=np.float32)
    Wout = np.asarray(Wout, dtype=np.float32)
    lam_v = (math.exp(float(np.dot(lambda_q1, lambda_k1)))
             - math.exp(float(np.dot(lambda_q2, lambda_k2))) + LAMBDA_INIT)
    lam_arr = np.full((2, 1), lam_v, dtype=np.float32)
    gam_arr = np.tile(
        (np.asarray(gamma, dtype=np.float32) * (1.0 - LAMBDA_INIT)), 2
    ).reshape(128, 1).copy()
    o2 = np.zeros((128, 2), dtype=bf)
    o2[0:64, 0] = 1.0
    o2[64:128, 1] = 1.0
    # bc: broadcast stationaries.  Row 0 cols 0:64 -> ones (K=1 broadcast of
    # the partition-0 r2 row to 64 partitions).  As a [2,128] K=2 stationary:
    # row 0 ones at cols 0:64, row 1 ones at cols 64:128 (rinv fan-out).
    bc_arr = np.zeros((2, 128), dtype=bf)
    bc_arr[0, 0:64] = 1.0
    bc_arr[1, 64:128] = 1.0
    idn = np.eye(128, dtype=np.float32).astype(bf)
    Wq_s = (Wq * (D ** -0.5)).astype(np.float32)
    Wk = Wkv[:, :E]
    Wv = Wkv[:, E:]

    def wtile(W, g):
        # [1024, 128] slice -> [128, 8*128] with [p, c*128+m] = W[c*128+p, m]
        ws = W[:, 128 * g:128 * (g + 1)]
        return np.ascontiguousarray(
            ws.reshape(8, 128, 128).transpose(1, 0, 2).reshape(128, 1024)
        ).astype(bf)

    xts = []
    for b in range(B):
        xb = x[b]  # [2048, 1024]
        a = xb.reshape(4, 512, 8, 128).transpose(0, 3, 2, 1)  # [tb,p,c,m]
        xts.append(np.ascontiguousarray(a.reshape(512, 4096)).astype(bf))

    in_maps = []
    for c in range(N_CORES):
        b, g = divmod(c, GROUPS)
        in_maps.append({
            "xt": xts[b],
            "wq": wtile(Wq_s, g),
            "wk": wtile(Wk, g),
            "wv": wtile(Wv, g),
            "wo": np.ascontiguousarray(
                Wout[128 * g:128 * (g + 1), :]).astype(bf),
            "lam2": lam_arr,
            "gamp": gam_arr,
            "ones2": o2,
            "bc": bc_arr,
            "idn": idn,
        })
    return in_maps


def kernel(x, Wq, Wkv, Wout, lambda_q1, lambda_k1, lambda_q2, lambda_k2,
           gamma, _run_kw=None):
    import sys
    if "/opt/trn_rl_repo" not in sys.path:
        sys.path.insert(0, "/opt/trn_rl_repo")
    from concourse.bass_utils import run_bass_kernel_spmd

    nc = _get_compiled()
    in_maps = make_in_maps(x, Wq, Wkv, Wout, lambda_q1, lambda_k1,
                           lambda_q2, lambda_k2, gamma)
    res = run_bass_kernel_spmd(nc, in_maps, list(range(N_CORES)),
                               **(_run_kw or {}))
    out = np.zeros((B, T, OUT_DIM), dtype=np.float32)
    for c in range(N_CORES):
        r = np.asarray(res.results[c]["outT"], dtype=np.float32)
        part = r.reshape(4, 128, 8, 512).transpose(0, 3, 2, 1).reshape(T, OUT_DIM)
        out[c // GROUPS] += part
    kernel.last_result = res
    return out


# revision 33
# speedup vs baseline: 1.0209x; 1.0209x over previous
"""DiffAttn Trainium2 kernel (8-core SPMD, no collectives) — v5.

Problem: B=2, T=2048, IN_DIM=OUT_DIM=1024, H=8 v-heads (2D=64), 2H=16 qk-heads
(D=32).  Core c = 4*b + g handles batch b, head-group g: qk-heads {4g..4g+3},
v-heads {2g, 2g+1}, all T queries of its batch.  Out-projection row-sharded;
host sums 4 partials per batch (bf16 partials; host upcasts and sums fp32).

Design notes (measured ~231us vs 242.5us for v2):
- The Act engine's exp stream (128 x [128,1024] ~1.12us each) is the pacing
  floor; PSUM (8 banks: accA+accB 4, "d" ring 2x4KB) limits Act lookahead
  to one tile, so every engine must keep dots flowing on schedule.
- In-order engine queues are the main hazard: never put a blocked
  instruction ahead of ready work.  dots are issued first each step;
  boundary drains stay off the Act queue mid-run; a heavy deferred
  projection borrows its step's attn@v slot (attn@v is elastic via the
  16-deep e ring).
- The PE DVFS clock (2.4 -> 1.2 GHz) drops on idle windows and everything
  downstream slows: warmup matmuls ramp it during the input DMA, qT2/qT3
  projections are scheduled INTO the qb-boundary holds, and dummy matmuls
  into the freed acc banks keep it up through the tail out-projection.
- Phase C: denominator rows (PSUM partition 64) land on partition 0 of one
  [1,2048] SBUF row via cross-partition DVE copies (legal when the start
  partition is 32-aligned; PSUM reads must also start 32-aligned and
  GPSIMD cannot touch PSUM at all).  r2/rinv fan out through tiny K=1/K=2
  PE matmuls against ones/mask stationaries (the old sA/sB DMA fan and 4
  gpsimd partition_broadcasts are gone).  ss is ONE matmul with a [128,2]
  ones stationary -> [2,512]; Ln/Exp run at 512-free; Square rides DVE
  (Act for the last qb); comb's bottom half is written cross-partition
  (no SBUF->SBUF DMA hop); the last qb's drains ride the idle Act engine
  and attn@v for its final key-tiles issues with no lag.
"""
import math

import numpy as np

H = 8
D = 32
LAMBDA_INIT = 0.8 - 0.6 * math.exp(-0.3)
B, T, IN_DIM, OUT_DIM = 2, 2048, 1024, 1024
E = 2 * H * D  # 512

N_CORES = 8
GROUPS = 4            # head groups (cores per batch)
QB = 512              # query block
NQB = T // QB         # 4
KT = 128              # key tile (PSUM partition dim for dots)
NKT = T // KT         # 16
NIN = IN_DIM // 128   # 8
LAG = 3               # attn@v issue lag in key-tiles
WARM = 10             # PE warmup matmuls (clock ramp during x DMA)

_compiled = None


def _build():
    import concourse.bass as bass  # noqa: F401
    import concourse.mybir as mybir
    from concourse import bacc
    from concourse.tile import TileContext

    f32 = mybir.dt.float32
    bf16 = mybir.dt.bfloat16
    AF = mybir.ActivationFunctionType
    MUL = mybir.AluOpType.mult

    nc = bacc.Bacc("TRN2", target_bir_lowering=False, num_devices=N_CORES)

    xt = nc.dram_tensor("xt", [4 * 128, NIN * 512], bf16, kind="ExternalInput")
    wq = nc.dram_tensor("wq", [128, NIN * 128], bf16, kind="ExternalInput")
    wk = nc.dram_tensor("wk", [128, NIN * 128], bf16, kind="ExternalInput")
    wv = nc.dram_tensor("wv", [128, NIN * 128], bf16, kind="ExternalInput")
    wo = nc.dram_tensor("wo", [128, OUT_DIM], bf16, kind="ExternalInput")
    lam2 = nc.dram_tensor("lam2", [2, 1], f32, kind="ExternalInput")
    gamp = nc.dram_tensor("gamp", [128, 1], f32, kind="ExternalInput")
    ones2 = nc.dram_tensor("ones2", [128, 2], bf16, kind="ExternalInput")
    bc = nc.dram_tensor("bc", [2, 128], bf16, kind="ExternalInput")
    idn = nc.dram_tensor("idn", [128, 128], bf16, kind="ExternalInput")
    outT = nc.dram_tensor("outT", [4 * 128, 8 * 512], bf16,
                          kind="ExternalOutput")

    with TileContext(nc) as tc:
        with tc.tile_pool(name="persist", bufs=1) as pp:
            # ---- persistent SBUF ----
            wq_sb = pp.tile([128, NIN * 128], bf16)
            wk_sb = pp.tile([128, NIN * 128], bf16)
            wv_sb = pp.tile([128, NIN * 128], bf16)
            wo_sb = pp.tile([128, OUT_DIM], bf16)
            xT_sb = pp.tile([128, 4, NIN * 512], bf16)
            # per-token-block tiles: deferred projections write only their
            # own block, so dots/attn@v never pick up false whole-tile deps
            qT_tb = [pp.tile([128, 512], bf16, name=f"qT{t}") for t in range(4)]
            kT_tb = [pp.tile([128, 512], bf16, name=f"kT{t}") for t in range(4)]
            vT_tb = [pp.tile([128, 512], bf16, name=f"vT{t}") for t in range(4)]
            v_tb = [pp.tile([128, 4, 130], bf16, name=f"v{t}") for t in range(4)]
            lam_sb = pp.tile([2, 1], f32)
            gam_sb = pp.tile([128, 1], f32)
            ones2_sb = pp.tile([128, 2], bf16)
            bc_sb = pp.tile([2, 128], bf16)
            idn_sb = pp.tile([128, 128], bf16)
            warm = pp.tile([128, 512], bf16)
            warm_e = pp.tile([128, 512], bf16)

            # x tb0 right after the k/q weights so the first projections can
            # start ~4us in; the rest interleaved with the small consts
            nc.sync.dma_start(out=wk_sb[:, :], in_=wk[:, :])
            nc.sync.dma_start(out=wq_sb[:, :], in_=wq[:, :])
            nc.sync.dma_start(out=xT_sb[:, 0, :], in_=xt[0:128, :])
            nc.sync.dma_start(out=idn_sb[:, :], in_=idn[:, :])
            nc.sync.dma_start(out=wv_sb[:, :], in_=wv[:, :])
            nc.sync.dma_start(out=xT_sb[:, 1, :], in_=xt[128:256, :])
            nc.sync.dma_start(out=lam_sb[:, :], in_=lam2[:, :])
            nc.sync.dma_start(out=gam_sb[:, :], in_=gamp[:, :])
            nc.sync.dma_start(out=ones2_sb[:, :], in_=ones2[:, :])
            nc.sync.dma_start(out=bc_sb[:, :], in_=bc[:, :])
            nc.sync.dma_start(out=wo_sb[:, :], in_=wo[:, :])
            nc.sync.dma_start(out=xT_sb[:, 2, :], in_=xt[256:384, :])
            nc.sync.dma_start(out=xT_sb[:, 3, :], in_=xt[384:512, :])

            # ---- phase A: warmup + projections ----
            with tc.tile_pool(name="psA", bufs=1, space="PSUM") as psA:
                nc.vector.memset(warm[:, :], 0.0)
                wm = psA.tile([128, 512], f32, tag="warm", bufs=1)
                for _ in range(WARM):
                    nc.tensor.matmul(wm[:, :], warm[:, :128], warm[:, :],
                                     start=True, stop=True)
                # natural_log_exp_and_others holds Exp+Copy+Ln: one load
                # up front keeps the Act table warm for the whole kernel
                nc.scalar.add_instruction(mybir.InstLoadActFuncSet(
                    name=nc.scalar.bass.get_next_instruction_name(),
                    ins=[], outs=[], act_func_set_id=6))
                nc.scalar.activation(warm_e[:, :], wm[:, :], AF.Exp)

                # minimal prefix: only what dots(qb0,kt0) needs — the kT0
                # and qT0 projections.  vT0 + its transposes are deferred
                # into the main loop (steps 0-1), ahead of attn@v(0) at
                # step LAG.  First exp ~14us instead of ~24us.
                for dst, w_sb in ((kT_tb[0], wk_sb), (qT_tb[0], wq_sb)):
                    p = psA.tile([128, 512], f32, tag="proj", bufs=2)
                    for c in range(NIN):
                        nc.tensor.matmul(
                            p[:, :], w_sb[:, 128 * c:128 * (c + 1)],
                            xT_sb[:, 0, 512 * c:512 * (c + 1)],
                            start=(c == 0), stop=(c == NIN - 1))
                    nc.vector.tensor_copy(dst[:, :], p[:, :])
                with tc.tile_pool(name="onescr", bufs=1) as op_:
                    oscr = op_.tile([128, 4], f32)
                    nc.vector.memset(oscr[:, :], 1.0)
                    for t in range(4):
                        nc.vector.tensor_copy(
                            v_tb[t][:, :, 64:65].rearrange("p n 1 -> p n"),
                            oscr[:, :])
                        nc.vector.tensor_copy(
                            v_tb[t][:, :, 129:130].rearrange("p n 1 -> p n"),
                            oscr[:, :])

            # ---- phase B + C interleaved ----
            with (
                tc.tile_pool(name="mp", bufs=1, space="PSUM") as mp,
                tc.tile_pool(name="ep", bufs=1) as ep,
                tc.tile_pool(name="cp", bufs=1) as cp,
            ):
                es = {}
                accs = {}
                pend_attnv = []   # (qb, kt) awaiting attn@v issue
                chunks = []       # pending phase-C closures
                hold = [0]        # skip-pops pending after a boundary

                def a_proj(dst, w_sb, tb, nm=[0], lo=0, hi=512):
                    nm[0] += 1
                    pname = f"aproj{nm[0]}"
                    w = hi - lo
                    def f():
                        p = mp.tile([128, 1024], f32, tag="d",
                                    name=pname, bufs=2)
                        for c in range(NIN):
                            nc.tensor.matmul(
                                p[:, 0:w], w_sb[:, 128 * c:128 * (c + 1)],
                                xT_sb[:, tb, 512 * c + lo:512 * c + hi],
                                start=(c == 0), stop=(c == NIN - 1))
                        nc.vector.tensor_copy(dst[tb][:, lo:hi], p[:, 0:w])
                    return f

                def a_transp(tb):
                    def f():
                        tp = mp.tile([128, 2048], bf16, tag="d",
                                     name=f"atr{tb}", bufs=2)
                        for j in range(4):
                            nc.tensor.transpose(
                                tp[:, 128 * j:128 * (j + 1)],
                                vT_tb[tb][:, 128 * j:128 * (j + 1)],
                                idn_sb[:, :])
                        for j in range(4):
                            nc.vector.tensor_copy(
                                v_tb[tb][:, j, 0:64],
                                tp[:, 128 * j:128 * j + 64])
                            nc.vector.tensor_copy(
                                v_tb[tb][:, j, 65:129],
                                tp[:, 128 * j + 64:128 * (j + 1)])
                    return f

                # (fn, is_heavy): heavy projections borrow the step's
                # attn@v PE slot; transposes are cheap enough not to.
                # qT2/qT3 sit ON the qb boundaries (steps 16/32) where the
                # hold idles the PE anyway — free real estate that also
                # keeps the DVFS clock up.
                awork = {
                    0: (a_proj(vT_tb, wv_sb, 0), True),
                    1: (a_transp(0), False),
                    2: (a_proj(kT_tb, wk_sb, 1), True),
                    4: (a_proj(vT_tb, wv_sb, 1), True),
                    5: (a_transp(1), False),
                    6: (a_proj(kT_tb, wk_sb, 2), True),
                    7: (a_proj(vT_tb, wv_sb, 2), True),
                    8: (a_transp(2), False),
                    9: (a_proj(kT_tb, wk_sb, 3), True),
                    10: (a_proj(vT_tb, wv_sb, 3), True),
                    11: (a_transp(3), False),
                    13: (a_proj(qT_tb, wq_sb, 1), True),
                    # qT2/qT3 in N=256 halves: one monolithic 8-matmul
                    # projection overruns a step's PE slack and delays the
                    # next dots (~4us exp gap); halves fit the boundary
                    # hold's slack
                    18: (a_proj(qT_tb, wq_sb, 2, lo=0, hi=256), False),
                    20: (a_proj(qT_tb, wq_sb, 2, lo=256, hi=512), False),
                    34: (a_proj(qT_tb, wq_sb, 3, lo=0, hi=256), False),
                    36: (a_proj(qT_tb, wq_sb, 3, lo=256, hi=512), False),
                }

                def issue_attnv(qb, kt):
                    if kt == 0:
                        accs[(qb, 0)] = mp.tile([65, 1024], f32, tag="accA",
                                                name=f"accA{qb}", bufs=1)
                        accs[(qb, 1)] = mp.tile([65, 1024], f32, tag="accB",
                                                name=f"accB{qb}", bufs=1)
                    for vh in range(2):
                        e = es.pop((qb, kt, vh))
                        for hh in range(2):
                            nc.tensor.matmul(
                                accs[(qb, vh)][:, 512 * hh:512 * (hh + 1)],
                                v_tb[kt // 4][:, kt % 4, 65 * vh:65 * (vh + 1)],
                                e[:, 512 * hh:512 * (hh + 1)],
                                start=(kt == 0), stop=(kt == NKT - 1))

                def make_chunks(qb):
                    aA, aB = accs.pop((qb, 0)), accs.pop((qb, 1))
                    last = qb == NQB - 1
                    a_sbA = cp.tile([64, 1024], f32, tag="asbA",
                                    name=f"asbA{qb}", bufs=2)
                    a_sbB = cp.tile([64, 1024], f32, tag="asbB",
                                    name=f"asbB{qb}", bufs=2)
                    srS = cp.tile([1, 2048], f32, tag="srS", name=f"srS{qb}",
                                  bufs=2)
                    # boundary drains (eager, so the acc banks free fast).
                    # denominator rows (PSUM partition 64) land on
                    # partition 0 via cross-partition copies.  Mid-run all
                    # four ride the DVE — putting any on Act would block
                    # the in-order exp stream behind attnv(qb,15).  Rows
                    # first so the ratio chain starts at +2.3us.  On the
                    # last qb the exp stream is over: Act takes the B side
                    # and the drains pair up.
                    nc.vector.tensor_copy(srS[0:1, 0:1024], aA[64:65, :])
                    if last:
                        nc.scalar.copy(srS[0:1, 1024:2048], aB[64:65, :])
                        nc.vector.tensor_copy(a_sbA[:, :], aA[0:64, :])
                        nc.scalar.copy(a_sbB[:, :], aB[0:64, :])
                    else:
                        nc.vector.tensor_copy(srS[0:1, 1024:2048],
                                              aB[64:65, :])
                        nc.vector.tensor_copy(a_sbA[:, :], aA[0:64, :])
                        nc.vector.tensor_copy(a_sbB[:, :], aB[0:64, :])

                    st = {}
                    nwt = [0]

                    def c_rcp():
                        rr = cp.tile([1, 1024], f32, tag="rr", bufs=2)
                        nc.vector.reciprocal_approx_fast(
                            out=rr[0:1, 0:512], in_=srS[0:1, 512:1024])
                        nc.vector.reciprocal_approx_fast(
                            out=rr[0:1, 512:1024], in_=srS[0:1, 1536:2048])
                        st["rr"] = rr

                    def c_r2():
                        r2 = cp.tile([1, 1024], bf16, tag="r2", bufs=2)
                        nc.vector.scalar_tensor_tensor(
                            r2[0:1, 0:512], srS[0:1, 0:512],
                            lam_sb[0:1, 0:1], st["rr"][0:1, 0:512],
                            op0=MUL, op1=MUL)
                        nc.vector.scalar_tensor_tensor(
                            r2[0:1, 512:1024], srS[0:1, 1024:1536],
                            lam_sb[0:1, 0:1], st["rr"][0:1, 512:1024],
                            op0=MUL, op1=MUL)
                        st["r2"] = r2

                    def tail_warm(n):
                        # last-qb only: dummy matmuls into the freed acc
                        # banks keep the PE clock from dropping before the
                        # out-projection
                        if not last:
                            return
                        wt = mp.tile([65, 512], f32, tag="accA",
                                     name=f"wt{qb}_{nwt[0]}", bufs=1)
                        nwt[0] += 1
                        for _ in range(n):
                            nc.tensor.matmul(wt[0:128 - 64, :],
                                             warm[:, 0:64], warm[:, :],
                                             start=True, stop=True)

                    def c_rb():
                        # broadcast the r2 row (partition 0) to 64
                        # partitions with two K=1 matmuls (ones stationary)
                        rb = mp.tile([64, 1024], f32, tag="d",
                                     name=f"rb{qb}", bufs=2)
                        for half in range(2):
                            nc.tensor.matmul(
                                rb[:, 512 * half:512 * (half + 1)],
                                bc_sb[0:1, 0:64],
                                st["r2"][0:1, 512 * half:512 * (half + 1)],
                                start=True, stop=True)
                        st["rb"] = rb
                        tail_warm(5)

                    def c_comb0():
                        t2 = cp.tile([64, 512], f32, tag="t2a", bufs=1)
                        comb = cp.tile([128, 512], f32, tag="comb",
                                       name=f"comb{qb}", bufs=2)
                        nc.vector.tensor_mul(t2[:, :],
                                             a_sbA[0:64, 512:1024],
                                             st["rb"][:, 0:512])
                        nc.vector.tensor_sub(comb[0:64, :],
                                             a_sbA[0:64, 0:512], t2[:, :])
                        st["comb"] = comb

                    def c_comb1():
                        # cross-partition write: in p0-63, out p64-127
                        # (legal: start partitions are 32-aligned) — no
                        # SBUF->SBUF DMA hop needed for the bottom half.
                        # On the last qb the pair rides GpSimd so the A
                        # and B halves run in parallel in the tail.
                        t2 = cp.tile([64, 512], f32, tag="t2b", bufs=1)
                        nc.vector.tensor_mul(t2[:, :],
                                             a_sbB[0:64, 512:1024],
                                             st["rb"][:, 512:1024])
                        nc.vector.tensor_sub(st["comb"][64:128, :],
                                             a_sbB[0:64, 0:512], t2[:, :])

                    def c_sq():
                        sq = cp.tile([128, 512], bf16, tag="sq", bufs=2)
                        if last:
                            nc.scalar.activation(sq[:, :], st["comb"][:, :],
                                                 AF.Square)
                        else:
                            nc.vector.tensor_mul(sq[:, :], st["comb"][:, :],
                                                 st["comb"][:, :])
                        st["sq"] = sq

                    def c_ss():
                        sst = mp.tile([2, 512], f32, tag="d",
                                      name=f"ss{qb}", bufs=2)
                        nc.tensor.matmul(sst[:, :], ones2_sb[:, 0:2],
                                         st["sq"][:, :], start=True, stop=True)
                        st["ss"] = sst
                        tail_warm(4)

                    def c_rinv():
                        rln = cp.tile([2, 512], f32, tag="rln", bufs=2)
                        nc.scalar.activation(rln[:, :], st["ss"][0:2, :],
                                             AF.Ln, scale=1.0 / 64.0)
                        rinv = cp.tile([2, 512], bf16, tag="rinv", bufs=2)
                        nc.scalar.activation(rinv[:, :], rln[:, :], AF.Exp,
                                             scale=-0.5)
                        st["rinv"] = rinv

                    def c_rb2():
                        # broadcast rinv rows (partitions 0/1) to 128
                        # partitions: top 64 get row 0, bottom 64 row 1
                        rb2 = mp.tile([128, 512], f32, tag="d",
                                      name=f"rb2_{qb}", bufs=2)
                        nc.tensor.matmul(rb2[:, :], bc_sb[0:2, 0:128],
                                         st["rinv"][0:2, :],
                                         start=True, stop=True)
                        st["rb2"] = rb2
                        tail_warm(4)

                    def c_finl():
                        finl = cp.tile([128, 512], bf16, tag="finl",
                                       name=f"finl{qb}", bufs=2)
                        nc.vector.scalar_tensor_tensor(
                            finl[:, :], st["comb"][:, :], gam_sb[:, 0:1],
                            st["rb2"][:, :], op0=MUL, op1=MUL)
                        st["finl"] = finl

                    def c_opj(p):
                        def f():
                            opj = mp.tile([128, 1024], f32, tag="d",
                                          name=f"opj{qb}_{p}", bufs=2)
                            for j in range(2):
                                oc = 2 * p + j
                                nc.tensor.matmul(
                                    opj[:, 512 * j:512 * (j + 1)],
                                    wo_sb[:, 128 * oc:128 * (oc + 1)],
                                    st["finl"][:, :], start=True, stop=True)
                            ostg = cp.tile([128, 1024], bf16, tag="ostg",
                                           name=f"ostg{qb}_{p}", bufs=2)
                            if last and p % 2 == 1:
                                nc.scalar.copy(ostg[:, :], opj[:, :])
                            else:
                                nc.vector.tensor_copy(ostg[:, :], opj[:, :])
                            nc.sync.dma_start(
                                out=outT[128 * qb:128 * (qb + 1),
                                         1024 * p:1024 * (p + 1)],
                                in_=ostg[:, :])
                        return f

                    return [c_rcp, c_r2, c_rb, c_comb0, c_comb1, c_sq,
                            c_ss, c_rinv, c_rb2, c_finl,
                            c_opj(0), None, c_opj(1), None,
                            c_opj(2), None, c_opj(3)]

                for qb in range(NQB):
                    for kt in range(NKT):
                        # all four dots back-to-back into 4 distinct PSUM
                        # banks -> 4-band row-tile concurrency on the PE
                        ds = [mp.tile([128, 1024], f32, tag="d",
                                      name=f"d{qb}_{kt}_{vh}", bufs=2)
                              for vh in range(2)]
                        for h in range(4):
                            nc.tensor.matmul(
                                ds[h // 2][:, 512 * (h % 2):512 * (h % 2 + 1)],
                                kT_tb[kt // 4][32 * h:32 * (h + 1),
                                               KT * (kt % 4):KT * (kt % 4 + 1)],
                                qT_tb[qb][32 * h:32 * (h + 1), :],
                                start=True, stop=True,
                                tile_position=(32 * h, 0))
                        for vh in range(2):
                            e = ep.tile([128, 1024], bf16, tag="e",
                                        name=f"e{qb}_{kt}_{vh}", bufs=16)
                            nc.scalar.activation(e[:, :], ds[vh][:, :], AF.Exp)
                            es[(qb, kt, vh)] = e
                        pend_attnv.append((qb, kt))
                        aw = awork.pop(16 * qb + kt, None)
                        if aw is not None:
                            aw[0]()
                            # a heavy projection borrows this step's
                            # attn@v PE slot (attn@v is elastic via the
                            # e-buf ring; delayed dots would gap the exps)
                            if aw[1] and not hold[0]:
                                hold[0] = 1
                        # final qb: no more exps to pace — issue attn@v
                        # immediately so the tail drain shrinks
                        lag_now = 0 if qb == NQB - 1 and kt >= NKT - 3 \
                            else LAG
                        if hold[0]:
                            hold[0] -= 1
                        else:
                            npops = 0
                            while len(pend_attnv) > lag_now and npops < 2:
                                aqb, akt = pend_attnv.pop(0)
                                issue_attnv(aqb, akt)
                                npops += 1
                                if akt == NKT - 1:
                                    chunks.extend(make_chunks(aqb))
                                    # give the boundary drain two extra kt
                                    # before the next qb's first attn@v
                                    # needs the acc banks back (the DVE
                                    # drain chain is ~4.6us ~ 2 kt)
                                    hold[0] = 3
                                    break
                        npop = 2 if len(chunks) > 10 else 1
                        for _ in range(npop):
                            if chunks:
                                ck = chunks.pop(0)
                                if ck is not None:
                                    ck()
                # drain
                while pend_attnv:
                    aqb, akt = pend_attnv.pop(0)
                    issue_attnv(aqb, akt)
                    if akt == NKT - 1:
                        chunks.extend(make_chunks(aqb))
                while chunks:
                    ck = chunks.pop(0)
                    if ck is not None:
                        ck()

    nc.compile()
    return nc


def _get_compiled():
    global _compiled
    if _compiled is None:
        _compiled = _build()
    return _compiled


def make_in_maps(x, Wq, Wkv, Wout, lambda_q1, lambda_k1, lambda_q2, lambda_k2,
                 gamma):
    import ml_dtypes
    bf = ml_dtypes.bfloat16
    x = np.asarray(x, dtype=np.float32)
    Wq = np.asarray(Wq, dtype=np.float32)
    Wkv = np.asarray(Wkv, dtype=np.float32)
    Wout = np.asarray(Wout, dtype=np.float32)
    lam_v = (math.exp(float(np.dot(lambda_q1, lambda_k1)))
             - math.exp(float(np.dot(lambda_q2, lambda_k2))) + LAMBDA_INIT)
    lam_arr = np.full((2, 1), lam_v, dtype=np.float32)
    gam_arr = np.tile(
        (np.asarray(gamma, dtype=np.float32) * (1.0 - LAMBDA_INIT)), 2
    ).reshape(128, 1).copy()
    o2 = np.zeros((128, 2), dtype=bf)
    o2[0:64, 0] = 1.0
    o2[64:128, 1] = 1.0
    # bc: broadcast stationaries.  Row 0 cols 0:64 -> ones (K=1 broadcast of
    # the partition-0 r2 row to 64 partitions).  As a [2,128] K=2 stationary:
    # row 0 ones at cols 0:64, row 1 ones at cols 64:128 (rinv fan-out).
    bc_arr = np.zeros((2, 128), dtype=bf)
    bc_arr[0, 0:64] = 1.0
    bc_arr[1, 64:128] = 1.0
    idn = np.eye(128, dtype=np.float32).astype(bf)
    Wq_s = (Wq * (D ** -0.5)).astype(np.float32)
    Wk = Wkv[:, :E]
    Wv = Wkv[:, E:]

    def wtile(W, g):
        # [1024, 128] slice -> [128, 8*128] with [p, c*128+m] = W[c*128+p, m]
        ws = W[:, 128 * g:128 * (g + 1)]
        return np.ascontiguousarray(
            ws.reshape(8, 128, 128).transpose(1, 0, 2).reshape(128, 1024)
        ).astype(bf)

    xts = []
    for b in range(B):
        xb = x[b]  # [2048, 1024]
        a = xb.reshape(4, 512, 8, 128).transpose(0, 3, 2, 1)  # [tb,p,c,m]
        xts.append(np.ascontiguousarray(a.reshape(512, 4096)).astype(bf))

    in_maps = []
    for c in range(N_CORES):
        b, g = divmod(c, GROUPS)
        in_maps.append({
            "xt": xts[b],
            "wq": wtile(Wq_s, g),
            "wk": wtile(Wk, g),
            "wv": wtile(Wv, g),
            "wo": np.ascontiguousarray(
                Wout[128 * g:128 * (g + 1), :]).astype(bf),
            "lam2": lam_arr,
            "gamp": gam_arr,
            "ones2": o2,
            "bc": bc_arr,
            "idn": idn,
        })
    return in_maps


def kernel(x, Wq, Wkv, Wout, lambda_q1, lambda_k1, lambda_q2, lambda_k2,
           gamma, _run_kw=None):
    import sys
    if "/opt/trn_rl_repo" not in sys.path:
        sys.path.insert(0, "/opt/trn_rl_repo")
    from concourse.bass_utils import run_bass_kernel_spmd

    nc = _get_compiled()
    in_maps = make_in_maps(x, Wq, Wkv, Wout, lambda_q1, lambda_k1,
                           lambda_q2, lambda_k2, gamma)
    res = run_bass_kernel_spmd(nc, in_maps, list(range(N_CORES)),
                               **(_run_kw or {}))
    out = np.zeros((B, T, OUT_DIM), dtype=np.float32)
    for c in range(N_CORES):
        r = np.asarray(res.results[c]["outT"], dtype=np.float32)
        part = r.reshape(4, 128, 8, 512).transpose(0, 3, 2, 1).reshape(T, OUT_DIM)
        out[c // GROUPS] += part
    kernel.last_result = res
    return out
